# revision 1
# baseline (speedup 1.0000x reference)
"""Trainium2 Bass kernel for nn_CrossPairMemory.

Sharding: data-parallel over batch across 8 NeuronCores (512 rows each),
weights replicated per core, no collectives.  All heavy matmuls run in
bf16 (fp32 PSUM accumulation); LayerNorm statistics and normalization in
fp32.  Activations are kept transposed (features on partitions, batch on
the free axis) through the fusion MLP so weight tiles act as the
stationary matmul operand in their natural HBM layout; the final
per-pair stage flips to activations-stationary so the output psum is
row-major and the last LayerNorm reduces along the free axis.
"""

import sys

for _p in ("/opt/trn_rl_repo",):
    if _p not in sys.path:
        sys.path.insert(0, _p)

import numpy as np
import ml_dtypes

import concourse.bass as bass
import concourse.tile as tile
from concourse import bacc, mybir
from concourse import bass_utils

BF = ml_dtypes.bfloat16
dt = mybir.dt
AF = mybir.ActivationFunctionType
ALU = mybir.AluOpType

NCORES = 8
B, P, PD, MD, S = 4096, 28, 128, 256, 64
D = P * PD            # 3584
K1T = 2 * P           # 56 contraction tiles for the first fusion matmul
Bc = B // NCORES      # 512 batch rows per core
# batch sub-chunks inside a core: small first chunk so its LN/gelu pass
# overlaps the second chunk's matmuls on the PE.
CHUNKS = ((0, 128), (128, 384))
EPS = 1e-5


def _bcast_ap(src_row):
    """Replicate a [N]-shaped dram AP across 128 partitions (stride-0)."""
    return bass.AP(
        tensor=src_row.tensor,
        offset=src_row.offset,
        ap=[[0, PD]] + [list(x) for x in src_row.ap],
    )


def _build():
    nc = bacc.Bacc(
        "TRN2", target_bir_lowering=False, debug=False, num_devices=NCORES
    )

    def din(name, shape, dty):
        return nc.dram_tensor(name, list(shape), dty, kind="ExternalInput").ap()

    psT = din("psT", (P, PD, Bc), dt.bfloat16)      # pair_states^T per pair
    msT = din("msT", (MD, Bc), dt.bfloat16)         # macro_state^T
    kP = din("kP", (PD, S), dt.bfloat16)            # pair keys^T, pre-scaled
    kM = din("kM", (MD, S), dt.bfloat16)            # macro keys^T, pre-scaled
    vP = din("vP", (S, D), dt.bfloat16)
    vM = din("vM", (S, D), dt.bfloat16)
    w1r = din("w1r", (P, PD, K1T, PD), dt.bfloat16)  # [n, kp, kt, f]
    w2r = din("w2r", (P, PD, P, PD), dt.bfloat16)    # [m, kp, kt, f]
    b1t = din("b1t", (PD, P), dt.float32)
    g1t = din("g1t", (PD, P), dt.float32)
    be1t = din("be1t", (PD, P), dt.float32)
    b2t = din("b2t", (PD, P), dt.float32)
    pwr = din("pwr", (PD, P, 2, PD), dt.bfloat16)    # [d, pair, ktile, e]
    pbr = din("pbr", (1, P, PD), dt.bfloat16)
    pgbc = din("pgbc", (PD, P, PD), dt.float32)      # ln_g broadcast rows
    pbbc = din("pbbc", (PD, P, PD), dt.float32)      # ln_b broadcast rows
    # output in [pair, btile, 128, 128] scratch layout: every DMA write is
    # one contiguous 64KB block; the host reassembles to (Bc, P, PD).
    out = nc.dram_tensor(
        "out", [P, Bc // PD, PD, PD], dt.float32, kind="ExternalOutput"
    ).ap()

    with tile.TileContext(nc) as tc:
        with (
            tc.tile_pool(name="const", bufs=1) as const,
            tc.tile_pool(name="res", bufs=1) as res,
        ):
            ones_col = const.tile([PD, 1], dt.bfloat16, tag="ones_col", name="ones_col")
            nc.vector.memset(ones_col, 1.0)
            ones_row_f = const.tile([1, PD], dt.float32, tag="ones_row_f", name="ones_row_f")
            nc.vector.memset(ones_row_f, 1.0)
            ones_row_b = const.tile([1, PD], dt.bfloat16, tag="ones_row_b", name="ones_row_b")
            nc.vector.memset(ones_row_b, 1.0)
            eps_t = const.tile([PD, 1], dt.float32, tag="eps", name="eps")
            nc.vector.memset(eps_t, EPS)

            lnc = {}
            for nm, src in (("b1", b1t), ("g1", g1t), ("be1", be1t), ("b2", b2t)):
                t = const.tile([PD, P], dt.float32, tag=f"lnc_{nm}", name=f"lnc_{nm}")
                nc.sync.dma_start(t, src)
                lnc[nm] = t
            pw_sb = const.tile([PD, P, 2, PD], dt.bfloat16, tag="pw_sb", name="pw_sb")
            nc.sync.dma_start(pw_sb, pwr)
            pb_sb = const.tile([1, P, PD], dt.bfloat16, tag="pb_sb", name="pb_sb")
            nc.sync.dma_start(pb_sb, pbr)

            # pair_states^T tiles stay resident: used by the score matmuls
            # (stage A) and again as stationary operands in stage C.
            psT_sb = []
            for p in range(P):
                t = res.tile([PD, Bc], dt.bfloat16, tag=f"psT{p}", name=f"psT{p}")
                nc.sync.dma_start(t, psT[p])
                psT_sb.append(t)

            with (
                tc.tile_pool(name="xt", bufs=1) as pxt,
                tc.tile_pool(name="h2", bufs=1) as ph2,
            ):
                xt_sb = [
                    pxt.tile([PD, Bc], dt.bfloat16, tag=f"xt{k}", name=f"xt{k}")
                    for k in range(K1T)
                ]
                h2_sb = [
                    ph2.tile([PD, Bc], dt.bfloat16, tag=f"h2{n}", name=f"h2{n}")
                    for n in range(P)
                ]

                # ---------------- stage A: associative memory reads --------
                with (
                    tc.tile_pool(name="stA", bufs=1) as pa,
                    tc.tile_pool(name="psA", bufs=2, space="PSUM") as ppa,
                    tc.tile_pool(name="psAc", bufs=2, space="PSUM") as ppac,
                ):
                    vP_sb = pa.tile([S, D], dt.bfloat16, tag="vP", name="vP")
                    nc.sync.dma_start(vP_sb, vP)
                    vM_sb = pa.tile([S, D], dt.bfloat16, tag="vM", name="vM")
                    nc.sync.dma_start(vM_sb, vM)
                    kP_sb = pa.tile([PD, S], dt.bfloat16, tag="kP", name="kP")
                    nc.sync.dma_start(kP_sb, kP)
                    kM0 = pa.tile([PD, S], dt.bfloat16, tag="kM0", name="kM0")
                    nc.sync.dma_start(kM0, kM[0:PD])
                    kM1 = pa.tile([PD, S], dt.bfloat16, tag="kM1", name="kM1")
                    nc.sync.dma_start(kM1, kM[PD:MD])
                    ms0 = pa.tile([PD, Bc], dt.bfloat16, tag="ms0", name="ms0")
                    nc.sync.dma_start(ms0, msT[0:PD])
                    ms1 = pa.tile([PD, Bc], dt.bfloat16, tag="ms1", name="ms1")
                    nc.sync.dma_start(ms1, msT[PD:MD])

                    def memory_read(which, vals_sb, xt_off):
                        sp = ppa.tile([S, Bc], dt.float32, tag="sp", name="sp")
                        if which == "pair":
                            for p in range(P):
                                nc.tensor.matmul(
                                    sp, kP_sb, psT_sb[p],
                                    start=(p == 0), stop=(p == P - 1),
                                )
                        else:
                            nc.tensor.matmul(sp, kM0, ms0, start=True, stop=False)
                            nc.tensor.matmul(sp, kM1, ms1, start=False, stop=True)
                        # scores are O(0.3): exp without max-subtraction is safe
                        eb = pa.tile([S, Bc], dt.bfloat16, tag=f"eb_{which}", name=f"eb_{which}")
                        nc.scalar.activation(eb, sp, AF.Exp)
                        den = ppa.tile([1, Bc], dt.float32, tag="den", name="den")
                        nc.tensor.matmul(den, ones_col[0:S, :], eb, start=True, stop=True)
                        rr = pa.tile([1, Bc], dt.float32, tag=f"rr_{which}", name=f"rr_{which}")
                        nc.vector.reciprocal(rr, den)
                        rbc = ppa.tile([S, Bc], dt.float32, tag="rbc", name="rbc")
                        nc.tensor.matmul(
                            rbc, ones_row_f[:, 0:S], rr, start=True, stop=True
                        )
                        ab = pa.tile([S, Bc], dt.bfloat16, tag=f"ab_{which}", name=f"ab_{which}")
                        nc.vector.tensor_mul(ab, eb, rbc)
                        for d in range(P):
                            pc = ppac.tile([PD, Bc], dt.float32, tag="pc", name="pc")
                            nc.tensor.matmul(
                                pc, vals_sb[:, d * PD:(d + 1) * PD], ab,
                                start=True, stop=True,
                            )
                            nc.scalar.activation(xt_sb[xt_off + d], pc, AF.Copy)

                    memory_read("pair", vP_sb, 0)
                    memory_read("macro", vM_sb, P)

                # ---------------- stage B: fusion MLP -----------------------
                with (
                    tc.tile_pool(name="hbf", bufs=1) as phb,
                    tc.tile_pool(name="psStat", bufs=1, space="PSUM") as ppst,
                ):
                    hbf = [
                        phb.tile([PD, Bc], dt.bfloat16, tag=f"hbf{n}", name=f"hbf{n}")
                        for n in range(P)
                    ]
                    stat_h = ppst.tile([1, Bc], dt.float32, tag="stat_h", name="stat_h")
                    stat_q = ppst.tile([1, Bc], dt.float32, tag="stat_q", name="stat_q")

                    with (
                        tc.tile_pool(name="w1s", bufs=2) as pw1,
                        tc.tile_pool(name="sqs", bufs=3) as psq,
                        tc.tile_pool(name="psM1", bufs=2, space="PSUM") as ppm1,
                    ):
                        for n in range(P):
                            w1b = pw1.tile([PD, K1T, PD], dt.bfloat16, tag="w1blk", name="w1blk")
                            nc.sync.dma_start(w1b, w1r[n])
                            for ci, (co, csz) in enumerate(CHUNKS):
                                pm = ppm1.tile([PD, csz], dt.float32, tag=f"pm{ci}", name=f"pm{ci}")
                                for k in range(K1T):
                                    nc.tensor.matmul(
                                        pm, w1b[:, k, :],
                                        xt_sb[k][:, co:co + csz],
                                        start=(k == 0), stop=(k == K1T - 1),
                                    )
                                nc.scalar.activation(
                                    hbf[n][:, co:co + csz], pm, AF.Identity,
                                    bias=lnc["b1"][:, n:n + 1], scale=1.0,
                                )
                            sq = psq.tile([PD, Bc], dt.bfloat16, tag="sq", name="sq")
                            nc.vector.tensor_mul(sq, hbf[n], hbf[n])
                            for co, csz in CHUNKS:
                                nc.tensor.matmul(
                                    stat_h[:, co:co + csz], ones_col,
                                    hbf[n][:, co:co + csz],
                                    start=(n == 0), stop=(n == P - 1),
                                    skip_group_check=True,
                                )
                                nc.tensor.matmul(
                                    stat_q[:, co:co + csz], ones_col,
                                    sq[:, co:co + csz],
                                    start=(n == 0), stop=(n == P - 1),
                                    skip_group_check=True,
                                )

                    # LayerNorm + gelu (per batch chunk)
                    with (
                        tc.tile_pool(name="lnrow", bufs=2) as plr,
                        tc.tile_pool(name="psBC", bufs=1, space="PSUM") as ppbc,
                        tc.tile_pool(name="tnorm", bufs=3) as ptn,
                    ):
                        for ci, (co, csz) in enumerate(CHUNKS):
                            cs = slice(co, co + csz)
                            mu_row = plr.tile([1, csz], dt.float32, tag=f"mu{ci}", name=f"mu{ci}")
                            nc.scalar.activation(
                                mu_row, stat_h[:, cs], AF.Copy, scale=1.0 / D
                            )
                            m2_row = plr.tile([1, csz], dt.float32, tag=f"m2{ci}", name=f"m2{ci}")
                            nc.scalar.activation(
                                m2_row, stat_q[:, cs], AF.Copy, scale=1.0 / D
                            )
                            var_row = plr.tile([1, csz], dt.float32, tag=f"va{ci}", name=f"va{ci}")
                            nc.vector.tensor_mul(var_row, mu_row, mu_row)
                            nc.vector.tensor_sub(var_row, m2_row, var_row)
                            sd_row = plr.tile([1, csz], dt.float32, tag=f"sd{ci}", name=f"sd{ci}")
                            nc.scalar.activation(
                                sd_row, var_row, AF.Sqrt,
                                bias=eps_t[0:1, :], scale=1.0,
                            )
                            rstd_row = plr.tile([1, csz], dt.float32, tag=f"rs{ci}", name=f"rs{ci}")
                            nc.vector.reciprocal(rstd_row, sd_row)
                            mu_bc = ppbc.tile([PD, csz], dt.float32, tag=f"mubc{ci}", name=f"mubc{ci}")
                            nc.tensor.matmul(
                                mu_bc, ones_row_f, mu_row, start=True, stop=True
                            )
                            rs_bc = ppbc.tile([PD, csz], dt.float32, tag=f"rsbc{ci}", name=f"rsbc{ci}")
                            nc.tensor.matmul(
                                rs_bc, ones_row_f, rstd_row, start=True, stop=True
                            )
                            for n in range(P):
                                t1 = ptn.tile([PD, csz], dt.float32, tag=f"t1_{ci}", name=f"t1_{ci}")
                                nc.vector.scalar_tensor_tensor(
                                    t1, hbf[n][:, cs], 1.0, mu_bc,
                                    op0=ALU.mult, op1=ALU.subtract,
                                )
                                t2 = ptn.tile([PD, csz], dt.float32, tag=f"t2_{ci}", name=f"t2_{ci}")
                                nc.vector.scalar_tensor_tensor(
                                    t2, t1, lnc["g1"][:, n:n + 1], rs_bc,
                                    op0=ALU.mult, op1=ALU.mult,
                                )
                                nc.scalar.activation(
                                    h2_sb[n][:, cs], t2, AF.Gelu,
                                    bias=lnc["be1"][:, n:n + 1], scale=1.0,
                                )

                # ------------- stage B2 + C: second matmul & per-pair -------
                with (
                    tc.tile_pool(name="w2s", bufs=2) as pw2,
                    tc.tile_pool(name="fus", bufs=3) as pfu,
                    tc.tile_pool(name="cbc", bufs=1) as pcb,
                    tc.tile_pool(name="scm", bufs=4) as psc,
                    tc.tile_pool(name="yout", bufs=3) as pyo,
                    tc.tile_pool(name="psM2", bufs=2, space="PSUM") as ppm2,
                    tc.tile_pool(name="psC", bufs=3, space="PSUM") as ppc,
                ):
                    gbc_sb = pcb.tile([PD, P, PD], dt.float32, tag="gbc_sb",
                                      name="gbc_sb")
                    nc.sync.dma_start(gbc_sb, pgbc)
                    bbc_sb = pcb.tile([PD, P, PD], dt.float32, tag="bbc_sb",
                                      name="bbc_sb")
                    nc.sync.dma_start(bbc_sb, pbbc)
                    for ci, (co, csz) in enumerate(CHUNKS):
                        cs = slice(co, co + csz)
                        for m in range(P):
                            w2b = pw2.tile([PD, P, PD], dt.bfloat16, tag="w2blk", name="w2blk")
                            nc.sync.dma_start(w2b, w2r[m])
                            pf = ppm2.tile([PD, csz], dt.float32, tag=f"pf{ci}", name=f"pf{ci}")
                            for k in range(P):
                                nc.tensor.matmul(
                                    pf, w2b[:, k, :], h2_sb[k][:, cs],
                                    start=(k == 0), stop=(k == P - 1),
                                )
                            fz = pfu.tile([PD, csz], dt.bfloat16, tag=f"fz{ci}", name=f"fz{ci}")
                            nc.scalar.activation(
                                fz, pf, AF.Identity,
                                bias=lnc["b2"][:, m:m + 1], scale=1.0,
                            )
                            gb = gbc_sb[:, m, :]
                            bb = bbc_sb[:, m, :]
                            for bt in range(csz // PD):
                                bs = slice(co + bt * PD, co + (bt + 1) * PD)
                                po = ppc.tile([PD, PD], dt.float32, tag="po", name="po")
                                nc.tensor.matmul(
                                    po, psT_sb[m][:, bs], pw_sb[:, m, 0, :],
                                    start=True, stop=False,
                                )
                                nc.tensor.matmul(
                                    po, fz[:, bt * PD:(bt + 1) * PD],
                                    pw_sb[:, m, 1, :],
                                    start=False, stop=False,
                                )
                                nc.tensor.matmul(
                                    po, ones_row_b, pb_sb[:, m, :],
                                    start=False, stop=True,
                                )
                                st6 = psc.tile([PD, 6], dt.float32, tag="st6", name="st6")
                                nc.vector.bn_stats(st6, po)
                                mv = psc.tile([PD, 2], dt.float32, tag="mv", name="mv")
                                nc.vector.bn_aggr(mv, st6)
                                sd2 = psc.tile([PD, 1], dt.float32, tag="sd2", name="sd2")
                                nc.scalar.activation(
                                    sd2, mv[:, 1:2], AF.Sqrt,
                                    bias=eps_t, scale=1.0,
                                )
                                rst2 = psc.tile([PD, 1], dt.float32, tag="rst2", name="rst2")
                                nc.vector.reciprocal(rst2, sd2)
                                tn = pyo.tile([PD, PD], dt.float32, tag="tn", name="tn")
                                nc.vector.tensor_scalar(
                                    tn, po, mv[:, 0:1], rst2,
                                    op0=ALU.subtract, op1=ALU.mult,
                                )
                                nc.vector.tensor_mul(tn, tn, gb)
                                y = pyo.tile([PD, PD], dt.float32, tag="y", name="y")
                                nc.vector.tensor_add(y, tn, bb)
                                nc.sync.dma_start(out[m, co // PD + bt], y)

    nc.compile()
    return nc


_CACHE = {}


def _get_nc():
    if "nc" not in _CACHE:
        _CACHE["nc"] = _build()
    return _CACHE["nc"]


def _prep_in_maps(inputs):
    f32 = np.float32
    g = lambda k: np.asarray(inputs[k], f32)

    psT_full = np.asarray(g("pair_states").transpose(1, 2, 0), dtype=BF)   # [P,PD,B]
    msT_full = np.asarray(g("macro_state").T, dtype=BF)                    # [MD,B]

    shared = {
        "kP": np.ascontiguousarray(
            (g("mem_pair_keys").T / (P * np.sqrt(PD))).astype(BF)),
        "kM": np.ascontiguousarray(
            (g("mem_macro_keys").T / np.sqrt(MD)).astype(BF)),
        "vP": g("mem_pair_vals").astype(BF),
        "vM": g("mem_macro_vals").astype(BF),
        "w1r": np.ascontiguousarray(
            g("fusion_w1").reshape(K1T, PD, P, PD).transpose(2, 1, 0, 3)
        ).astype(BF),
        "w2r": np.ascontiguousarray(
            g("fusion_w2").reshape(P, PD, P, PD).transpose(2, 1, 0, 3)
        ).astype(BF),
        "b1t": np.ascontiguousarray(g("fusion_b1").reshape(P, PD).T),
        "g1t": np.ascontiguousarray(g("fusion_ln_g").reshape(P, PD).T),
        "be1t": np.ascontiguousarray(g("fusion_ln_b").reshape(P, PD).T),
        "b2t": np.ascontiguousarray(g("fusion_b2").reshape(P, PD).T),
        "pwr": np.ascontiguousarray(
            g("pair_w").reshape(P, 2, PD, PD).transpose(2, 0, 1, 3)
        ).astype(BF),
        "pbr": g("pair_b").astype(BF).reshape(1, P, PD),
        "pgbc": np.ascontiguousarray(
            np.broadcast_to(g("pair_ln_g")[None], (PD, P, PD))),
        "pbbc": np.ascontiguousarray(
            np.broadcast_to(g("pair_ln_b")[None], (PD, P, PD))),
    }
    in_maps = []
    for c in range(NCORES):
        m = dict(shared)
        m["psT"] = np.ascontiguousarray(psT_full[:, :, c * Bc:(c + 1) * Bc])
        m["msT"] = np.ascontiguousarray(msT_full[:, c * Bc:(c + 1) * Bc])
        in_maps.append(m)
    return in_maps


def _run(inputs, trace=False):
    nc = _get_nc()
    in_maps = _prep_in_maps(inputs)
    res = bass_utils.run_bass_kernel_spmd(
        nc, in_maps, core_ids=list(range(NCORES)), trace=trace
    )
    # out scratch layout [P, Bc//PD, PD, PD] -> (Bc, P, PD) per core
    outp = np.concatenate(
        [
            res.results[c]["out"].transpose(1, 2, 0, 3).reshape(Bc, P, PD)
            for c in range(NCORES)
        ],
        axis=0,
    )
    return np.ascontiguousarray(outp.astype(np.float32)), res


def kernel(**inputs):
    outp, _ = _run(inputs, trace=False)
    return outp



# revision 2
# speedup vs baseline: 2.5637x; 2.5637x over previous
"""Trainium2 Bass kernel for nn_CrossPairMemory.

Sharding: data-parallel over batch across 8 NeuronCores (512 rows each),
weights replicated per core, no collectives.

Algebraic restructuring (all folds are weight-only, done host-side in fp32):
  * The fusion first Linear collapses through the associative memory read:
      h = [A_P | A_M] @ C,  C = [[vP @ W1_top + b1], [vM @ W1_bot]]
    where A_* are the (Bc, 64) attention matrices.  This removes the
    26 GFLOP/core (Bc,7168)x(7168,3584) matmul entirely.
  * LayerNorm-1 statistics come from the same algebra:
      sum_f h = c1^T a      with c1 = C.sum(axis=1)
      sum_f h^2 = a^T G a   with G = C @ C^T   (kept in fp32 on device)
    so h is never materialized pre-norm.
  * LayerNorm-1 apply is folded into the mm1 matmul: the attention matrix
    is scaled per-column by rstd, C is pre-scaled per-feature by ln_g, and
    the -mu*rstd*ln_g offset enters via a K=1 rank-1 matmul into the same
    PSUM accumulation; gelu(scale+bias) reads PSUM directly.
  * The second fusion Linear and the per-pair output Linear collapse:
      W2' = W2 @ blockdiag(pair_w[:,128:,:]),  b' = b2 @ blockdiag(..) + pair_b
    so one (Bc,3584)x(3584,3584) matmul plus a small pair_states @ pw_top
    term produces the pre-LN per-pair outputs directly, batch-major.
"""

import sys

for _p in ("/opt/trn_rl_repo",):
    if _p not in sys.path:
        sys.path.insert(0, _p)

import numpy as np
import ml_dtypes

import concourse.bass as bass
import concourse.tile as tile
from concourse import bacc, mybir
from concourse import bass_utils

BF = ml_dtypes.bfloat16
dt = mybir.dt
AF = mybir.ActivationFunctionType
ALU = mybir.AluOpType

NCORES = 8
B, P, PD, MD, S = 4096, 28, 128, 256, 64
D = P * PD            # 3584
Bc = B // NCORES      # 512 batch rows per core
NBT = Bc // PD        # 4 batch tiles of 128
MG = 7                # mm2 column groups of 4 pairs (512 cols)
EPS = 1e-5


def _build():
    nc = bacc.Bacc(
        "TRN2", target_bir_lowering=False, debug=False, num_devices=NCORES
    )

    def din(name, shape, dty):
        return nc.dram_tensor(name, list(shape), dty, kind="ExternalInput").ap()

    psT = din("psT", (P, PD, Bc), dt.bfloat16)      # pair_states^T per pair
    msT = din("msT", (2, PD, Bc), dt.bfloat16)      # macro_state^T, 2 tiles
    kP = din("kP", (PD, S), dt.bfloat16)            # pair keys^T, pre-scaled
    kM = din("kM", (2, PD, S), dt.bfloat16)         # macro keys^T, pre-scaled
    Cg = din("Cg", (PD, D), dt.bfloat16)            # C * ln1_g, slot-major
    c1 = din("c1", (PD, 1), dt.float32)             # C row-sums
    Gm = din("Gm", (PD, PD), dt.float32)            # C @ C^T
    grow = din("grow", (1, D), dt.bfloat16)         # ln1_g row
    be1t = din("be1t", (PD, P), dt.float32)         # ln1_b, feature-major
    w2p = din("w2p", (MG, PD, P, 4 * PD), dt.bfloat16)  # W2' blocks
    pwt = din("pwt", (PD, P, PD), dt.bfloat16)      # pair_w top half, d-major
    bprow = din("bprow", (1, D), dt.bfloat16)       # b2 @ pw_bot + pair_b
    g2bc = din("g2bc", (PD, P, PD), dt.float32)     # pair_ln_g broadcast
    b2bc = din("b2bc", (PD, P, PD), dt.float32)     # pair_ln_b broadcast
    out = nc.dram_tensor(
        "out", [Bc, D], dt.float32, kind="ExternalOutput"
    ).ap()

    with tile.TileContext(nc) as tc:
        with (
            tc.tile_pool(name="const", bufs=1) as const,
            tc.tile_pool(name="res", bufs=1) as res,
            tc.tile_pool(name="gres", bufs=1) as gres,
        ):
            ones_col_b = const.tile([PD, 1], dt.bfloat16, tag="ocb", name="ocb")
            nc.vector.memset(ones_col_b, 1.0)
            ones_col_f = const.tile([PD, 1], dt.float32, tag="ocf", name="ocf")
            nc.vector.memset(ones_col_f, 1.0)
            ones_row_b = const.tile([1, PD], dt.bfloat16, tag="orb", name="orb")
            nc.vector.memset(ones_row_b, 1.0)
            ones_row_f = const.tile([1, PD], dt.float32, tag="orf", name="orf")
            nc.vector.memset(ones_row_f, 1.0)
            eps_t = const.tile([PD, 1], dt.float32, tag="eps", name="eps")
            nc.vector.memset(eps_t, EPS)

            cst = {}
            for nm, src, shp, dty in (
                ("kP", kP, (PD, S), dt.bfloat16),
                ("Cg", Cg, (PD, D), dt.bfloat16),
                ("c1", c1, (PD, 1), dt.float32),
                ("Gm", Gm, (PD, PD), dt.float32),
                ("grow", grow, (1, D), dt.bfloat16),
                ("be1t", be1t, (PD, P), dt.float32),
                ("pwt", pwt, (PD, P, PD), dt.bfloat16),
                ("bprow", bprow, (1, D), dt.bfloat16),
                ("g2bc", g2bc, (PD, P, PD), dt.float32),
                ("b2bc", b2bc, (PD, P, PD), dt.float32),
            ):
                t = const.tile(list(shp), dty, tag=nm, name=nm)
                nc.sync.dma_start(t, src)
                cst[nm] = t
            kM_sb, ms_sb = [], []
            for i in range(2):
                t = const.tile([PD, S], dt.bfloat16, tag=f"kM{i}", name=f"kM{i}")
                nc.sync.dma_start(t, kM[i])
                kM_sb.append(t)
                t = const.tile([PD, Bc], dt.bfloat16, tag=f"ms{i}", name=f"ms{i}")
                nc.sync.dma_start(t, msT[i])
                ms_sb.append(t)

            # pair_states^T tiles stay resident: scores (stage A) + stage C.
            psT_sb = []
            for p in range(P):
                t = res.tile([PD, Bc], dt.bfloat16, tag=f"psT{p}", name=f"psT{p}")
                nc.sync.dma_start(t, psT[p])
                psT_sb.append(t)

            # post-gelu activations, feature-major k-tiles (mm2 stationary)
            gsb = [
                gres.tile([PD, Bc], dt.bfloat16, tag=f"g{n}", name=f"g{n}")
                for n in range(P)
            ]

            # ---------------- front: memory read + LN1 + gelu ----------
            with tc.tile_pool(name="fr", bufs=1) as fr:
                abPM = fr.tile([PD, Bc], dt.bfloat16, tag="abPM", name="abPM")
                abF = fr.tile([PD, Bc], dt.float32, tag="abF", name="abF")
                aprime = fr.tile([PD, Bc], dt.bfloat16, tag="apr", name="apr")
                negmr = fr.tile([1, Bc], dt.bfloat16, tag="negmr", name="negmr")

                with (
                    tc.tile_pool(name="psSp", bufs=2, space="PSUM") as ppsp,
                    tc.tile_pool(name="psBc", bufs=2, space="PSUM") as ppbc,
                    tc.tile_pool(name="psRw", bufs=2, space="PSUM") as pprw,
                ):
                    for which, off in (("pair", 0), ("macro", S)):
                        sp = ppsp.tile([S, Bc], dt.float32, tag="sp", name="sp")
                        if which == "pair":
                            for p in range(P):
                                nc.tensor.matmul(
                                    sp, cst["kP"], psT_sb[p],
                                    start=(p == 0), stop=(p == P - 1),
                                )
                        else:
                            nc.tensor.matmul(sp, kM_sb[0], ms_sb[0],
                                             start=True, stop=False)
                            nc.tensor.matmul(sp, kM_sb[1], ms_sb[1],
                                             start=False, stop=True)
                        # scores are tiny: exp without max-subtraction
                        eb = fr.tile([S, Bc], dt.bfloat16, tag=f"eb{off}",
                                     name=f"eb{off}")
                        nc.scalar.activation(eb, sp, AF.Exp)
                        den = pprw.tile([1, Bc], dt.float32, tag="den", name="den")
                        nc.tensor.matmul(den, ones_col_b[0:S, :], eb,
                                         start=True, stop=True)
                        rr = fr.tile([1, Bc], dt.float32, tag=f"rr{off}",
                                     name=f"rr{off}")
                        nc.vector.reciprocal(rr, den)
                        rbc = ppbc.tile([S, Bc], dt.float32, tag="rbc", name="rbc")
                        nc.tensor.matmul(rbc, ones_row_f[:, 0:S], rr,
                                         start=True, stop=True)
                        nc.vector.tensor_mul(abPM[off:off + S, :], eb, rbc)

                # f32 copy of attention for exact fp32 stats matmuls
                nc.scalar.activation(abF, abPM, AF.Copy)

                with tc.tile_pool(name="psSt", bufs=1, space="PSUM") as ppst:
                    murow = ppst.tile([1, Bc], dt.float32, tag="mu", name="mu")
                    nc.tensor.matmul(murow, cst["c1"], abF, start=True, stop=True)
                    Gt = ppst.tile([PD, Bc], dt.float32, tag="Gt", name="Gt")
                    nc.tensor.matmul(Gt, cst["Gm"], abF, start=True, stop=True)
                    qq = fr.tile([PD, Bc], dt.float32, tag="qq", name="qq")
                    nc.vector.tensor_mul(qq, abF, Gt)
                    sqrow = ppst.tile([1, Bc], dt.float32, tag="sq", name="sq")
                    nc.tensor.matmul(sqrow, ones_col_f, qq, start=True, stop=True)

                    m = fr.tile([1, Bc], dt.float32, tag="m", name="m")
                    nc.vector.tensor_scalar_mul(m, murow, 1.0 / D)
                    m2 = fr.tile([1, Bc], dt.float32, tag="m2", name="m2")
                    nc.vector.tensor_mul(m2, m, m)
                    var = fr.tile([1, Bc], dt.float32, tag="var", name="var")
                    nc.vector.scalar_tensor_tensor(
                        var, sqrow, 1.0 / D, m2, op0=ALU.mult, op1=ALU.subtract
                    )
                    sd = fr.tile([1, Bc], dt.float32, tag="sd", name="sd")
                    nc.scalar.activation(sd, var, AF.Sqrt,
                                         bias=eps_t[0:1, :], scale=1.0)
                    rstd = fr.tile([1, Bc], dt.float32, tag="rstd", name="rstd")
                    nc.vector.reciprocal(rstd, sd)
                    # negmr = -mu * rstd (bf16 row, K=1 matmul operand)
                    nc.vector.scalar_tensor_tensor(
                        negmr, m, -1.0, rstd, op0=ALU.mult, op1=ALU.mult
                    )
                    rstd_bc = ppst.tile([PD, Bc], dt.float32, tag="rbc2",
                                        name="rbc2")
                    nc.tensor.matmul(rstd_bc, ones_row_f, rstd,
                                     start=True, stop=True)
                    nc.vector.tensor_mul(aprime, abPM, rstd_bc)

                # mm1': per feature tile n, LN1+gelu fused via PSUM
                with tc.tile_pool(name="psM1", bufs=2, space="PSUM") as ppm1:
                    for n in range(P):
                        nsl = slice(n * PD, (n + 1) * PD)
                        pm = ppm1.tile([PD, Bc], dt.float32, tag="pm", name="pm")
                        nc.tensor.matmul(pm, cst["grow"][:, nsl], negmr,
                                         start=True, stop=False)
                        nc.tensor.matmul(pm, cst["Cg"][:, nsl], aprime,
                                         start=False, stop=True)
                        nc.scalar.activation(
                            gsb[n], pm, AF.Gelu,
                            bias=cst["be1t"][:, n:n + 1], scale=1.0,
                        )

            # ------------- stage BC: mm2' + pair_states part + LN2 ------
            with (
                tc.tile_pool(name="w2s", bufs=2) as pw2,
                tc.tile_pool(name="yo", bufs=3) as pyo,
                tc.tile_pool(name="sc", bufs=4) as psc,
                tc.tile_pool(name="psC", bufs=2, space="PSUM") as ppc,
            ):
                for mg in range(MG):
                    w2b = pw2.tile([PD, P, 4 * PD], dt.bfloat16, tag="w2b",
                                   name="w2b")
                    nc.sync.dma_start(w2b, w2p[mg])
                    mgsl = slice(mg * 4 * PD, (mg + 1) * 4 * PD)
                    for bt in range(NBT):
                        bs = slice(bt * PD, (bt + 1) * PD)
                        po = ppc.tile([PD, 4 * PD], dt.float32, tag="po",
                                      name="po")
                        nc.tensor.matmul(po, ones_row_b, cst["bprow"][:, mgsl],
                                         start=True, stop=False)
                        for s in range(4):
                            pidx = 4 * mg + s
                            nc.tensor.matmul(
                                po[:, s * PD:(s + 1) * PD],
                                psT_sb[pidx][:, bs], cst["pwt"][:, pidx, :],
                                start=False, stop=False, skip_group_check=True,
                            )
                        for k in range(P):
                            nc.tensor.matmul(
                                po, gsb[k][:, bs], w2b[:, k, :],
                                start=False, stop=(k == P - 1),
                                skip_group_check=True,
                            )
                        y4 = pyo.tile([PD, 4 * PD], dt.float32, tag="y4",
                                      name="y4")
                        for s in range(4):
                            pidx = 4 * mg + s
                            ssl = slice(s * PD, (s + 1) * PD)
                            st6 = psc.tile([PD, 6], dt.float32, tag="st6",
                                           name="st6")
                            nc.vector.bn_stats(st6, po[:, ssl])
                            mv = psc.tile([PD, 2], dt.float32, tag="mv",
                                          name="mv")
                            nc.vector.bn_aggr(mv, st6)
                            sd2 = psc.tile([PD, 1], dt.float32, tag="sd2",
                                           name="sd2")
                            nc.scalar.activation(sd2, mv[:, 1:2], AF.Sqrt,
                                                 bias=eps_t, scale=1.0)
                            rst2 = psc.tile([PD, 1], dt.float32, tag="rst2",
                                            name="rst2")
                            nc.vector.reciprocal(rst2, sd2)
                            tn = psc.tile([PD, PD], dt.float32, tag="tn",
                                          name="tn")
                            nc.vector.tensor_scalar(
                                tn, po[:, ssl], mv[:, 0:1], rst2,
                                op0=ALU.subtract, op1=ALU.mult,
                            )
                            tg = psc.tile([PD, PD], dt.float32, tag="tg",
                                          name="tg")
                            nc.vector.tensor_mul(tg, tn, cst["g2bc"][:, pidx, :])
                            nc.vector.tensor_add(
                                y4[:, ssl], tg, cst["b2bc"][:, pidx, :]
                            )
                        nc.sync.dma_start(out[bs, mgsl], y4)

    nc.compile()
    return nc


_CACHE = {}


def _get_nc():
    if "nc" not in _CACHE:
        _CACHE["nc"] = _build()
    return _CACHE["nc"]


def _prep_in_maps(inputs):
    f32 = np.float32
    g = lambda k: np.asarray(inputs[k], f32)

    psT_full = np.asarray(g("pair_states").transpose(1, 2, 0), dtype=BF)  # [P,PD,B]
    msT_full = np.asarray(g("macro_state").T, dtype=BF)                   # [MD,B]

    W1 = g("fusion_w1")                       # (7168, 3584)
    C = np.concatenate(
        [
            g("mem_pair_vals") @ W1[:D] + g("fusion_b1")[None, :],
            g("mem_macro_vals") @ W1[D:],
        ],
        axis=0,
    )                                          # (128, 3584)
    g1 = g("fusion_ln_g")
    pw = g("pair_w")                           # (28, 256, 128)
    pwA, pwB = pw[:, :PD, :], pw[:, PD:, :]
    # W2' = W2 @ blockdiag(pwB): (3584, 28, 128)
    W2r = g("fusion_w2").reshape(D, P, PD)
    W2p = np.matmul(W2r.transpose(1, 0, 2), pwB)          # (28, 3584, 128)
    W2p = W2p.transpose(1, 0, 2).reshape(D, D)
    bp = (
        np.einsum("pc,pce->pe", g("fusion_b2").reshape(P, PD), pwB)
        + g("pair_b")
    ).reshape(1, D)

    shared = {
        "kP": np.ascontiguousarray(
            (g("mem_pair_keys").T / (P * np.sqrt(PD))).astype(BF)),
        "kM": np.ascontiguousarray(
            (g("mem_macro_keys").T / np.sqrt(MD)).reshape(2, PD, S).astype(BF)),
        "Cg": np.ascontiguousarray((C * g1[None, :]).astype(BF)),
        "c1": np.ascontiguousarray(C.sum(axis=1, dtype=np.float64)
                                   .astype(f32).reshape(PD, 1)),
        "Gm": np.ascontiguousarray((C @ C.T).astype(f32)),
        "grow": np.ascontiguousarray(g1.reshape(1, D).astype(BF)),
        "be1t": np.ascontiguousarray(g("fusion_ln_b").reshape(P, PD).T),
        "w2p": np.ascontiguousarray(
            W2p.reshape(P, PD, MG, 4 * PD).transpose(2, 1, 0, 3).astype(BF)),
        "pwt": np.ascontiguousarray(pwA.transpose(1, 0, 2).astype(BF)),
        "bprow": np.ascontiguousarray(bp.astype(BF)),
        "g2bc": np.ascontiguousarray(
            np.broadcast_to(g("pair_ln_g")[None], (PD, P, PD))),
        "b2bc": np.ascontiguousarray(
            np.broadcast_to(g("pair_ln_b")[None], (PD, P, PD))),
    }
    in_maps = []
    for c in range(NCORES):
        m = dict(shared)
        m["psT"] = np.ascontiguousarray(psT_full[:, :, c * Bc:(c + 1) * Bc])
        m["msT"] = np.ascontiguousarray(
            msT_full[:, c * Bc:(c + 1) * Bc].reshape(2, PD, Bc))
        in_maps.append(m)
    return in_maps


def _run(inputs, trace=False):
    nc = _get_nc()
    in_maps = _prep_in_maps(inputs)
    res = bass_utils.run_bass_kernel_spmd(
        nc, in_maps, core_ids=list(range(NCORES)), trace=trace
    )
    outp = np.concatenate(
        [res.results[c]["out"] for c in range(NCORES)], axis=0
    ).reshape(B, P, PD)
    return np.ascontiguousarray(outp.astype(np.float32)), res


def kernel(**inputs):
    outp, _ = _run(inputs, trace=False)
    return outp


# revision 11
# speedup vs baseline: 3.2026x; 1.2492x over previous
"""Trainium2 Bass kernel for nn_CrossPairMemory.

Sharding: data-parallel over batch across 8 NeuronCores (512 rows each),
weights replicated per core, no collectives.

Algebraic restructuring (all folds are weight-only, done host-side in fp32):
  * The fusion first Linear collapses through the associative memory read:
      h = [A_P | A_M] @ C,  C = [[vP @ W1_top + b1], [vM @ W1_bot]]
    where A_* are the (Bc, 64) attention matrices.  This removes the
    26 GFLOP/core (Bc,7168)x(7168,3584) matmul entirely.
  * LayerNorm-1 statistics come from the same algebra:
      sum_f h = c1^T a      with c1 = C.sum(axis=1)
      sum_f h^2 = a^T G a   with G = C @ C^T   (kept in fp32 on device)
    so h is never materialized pre-norm.
  * LayerNorm-1 apply is folded into the mm1 matmul: the attention matrix
    is scaled per-column by rstd, C is pre-scaled per-feature by ln_g, and
    the -mu*rstd*ln_g offset enters via a K=1 rank-1 matmul into the same
    PSUM accumulation; gelu(scale+bias) reads PSUM directly.
  * The second fusion Linear and the per-pair output Linear collapse:
      W2' = W2 @ blockdiag(pair_w[:,128:,:]),  b' = b2 @ blockdiag(..) + pair_b
    so one (Bc,3584)x(3584,3584) matmul plus a small pair_states @ pw_top
    term produces the pre-LN per-pair outputs directly, batch-major.

Input-adaptive fast paths (checked on the actual arrays, general fallback):
skip the final LN scale/shift when pair_ln_g==1 and pair_ln_b==0, and skip
the stage-C bias matmul when the folded bias is exactly zero.
"""

import sys

for _p in ("/opt/trn_rl_repo",):
    if _p not in sys.path:
        sys.path.insert(0, _p)

import numpy as np
import ml_dtypes

import concourse.bass as bass
import concourse.tile as tile
from concourse import bacc, mybir
from concourse import bass_utils

BF = ml_dtypes.bfloat16
dt = mybir.dt
AF = mybir.ActivationFunctionType
ALU = mybir.AluOpType

NCORES = 8
B, P, PD, MD, S = 4096, 28, 128, 256, 64
D = P * PD            # 3584
Bc = B // NCORES      # 512 batch rows per core
NBT = Bc // PD        # 4 batch tiles of 128
MG = 7                # mm2 column groups of 4 pairs (512 cols)
EPS = 1e-5


def _build(unit_ln2, zero_bias):
    nc = bacc.Bacc(
        "TRN2", target_bir_lowering=False, debug=False, num_devices=NCORES
    )

    def din(name, shape, dty):
        return nc.dram_tensor(name, list(shape), dty, kind="ExternalInput").ap()

    psT = din("psT", (P, PD, Bc), dt.bfloat16)      # pair_states^T per pair
    msT = din("msT", (2, PD, Bc), dt.bfloat16)      # macro_state^T, 2 tiles
    kP = din("kP", (PD, S), dt.bfloat16)            # pair keys^T, pre-scaled
    kM = din("kM", (2, PD, S), dt.bfloat16)         # macro keys^T, pre-scaled
    Cg = din("Cg", (PD, D), dt.bfloat16)            # C * ln1_g, slot-major
    c1 = din("c1", (PD, 1), dt.float32)             # C row-sums
    Gm = din("Gm", (PD, PD), dt.float32)            # C @ C^T
    grow = din("grow", (1, D), dt.bfloat16)         # ln1_g row
    be1t = din("be1t", (PD, P), dt.float32)         # ln1_b, feature-major
    w2p = din("w2p", (MG, PD, P, 4 * PD), dt.bfloat16)  # W2' blocks
    pwt = din("pwt", (PD, P, PD), dt.bfloat16)      # pair_w top half, d-major
    if not zero_bias:
        bprow = din("bprow", (1, D), dt.bfloat16)   # b2 @ pw_bot + pair_b
    if not unit_ln2:
        g2bc = din("g2bc", (PD, P, PD), dt.float32)  # pair_ln_g broadcast
        b2bc = din("b2bc", (PD, P, PD), dt.float32)  # pair_ln_b broadcast
    out = nc.dram_tensor(
        "out", [Bc, D], dt.float32, kind="ExternalOutput"
    ).ap()

    with tile.TileContext(nc) as tc:
        with (
            tc.tile_pool(name="const", bufs=1) as const,
            tc.tile_pool(name="res", bufs=1) as res,
            tc.tile_pool(name="gres", bufs=1) as gres,
            tc.tile_pool(name="w2s", bufs=2) as pw2,
        ):
            ones_col_b = const.tile([PD, 1], dt.bfloat16, tag="ocb", name="ocb")
            nc.vector.memset(ones_col_b, 1.0)
            ones_col_f = const.tile([PD, 1], dt.float32, tag="ocf", name="ocf")
            nc.vector.memset(ones_col_f, 1.0)
            ones_row_b = const.tile([1, PD], dt.bfloat16, tag="orb", name="orb")
            nc.vector.memset(ones_row_b, 1.0)
            ones_row_f = const.tile([1, PD], dt.float32, tag="orf", name="orf")
            nc.vector.memset(ones_row_f, 1.0)
            eps_t = const.tile([PD, 1], dt.float32, tag="eps", name="eps")
            nc.vector.memset(eps_t, EPS)
            warm = const.tile([PD, Bc], dt.bfloat16, tag="warm", name="warm")
            nc.vector.memset(warm, 0.0)

            cst = {}

            def cload(nm, src, shp, dty):
                t = const.tile(list(shp), dty, tag=nm, name=nm)
                nc.sync.dma_start(t, src)
                cst[nm] = t

            # DMAs in consumption order: scores path first, stage C last.
            cload("kP", kP, (PD, S), dt.bfloat16)
            kM_sb, ms_sb = [], []
            for i in range(2):
                t = const.tile([PD, S], dt.bfloat16, tag=f"kM{i}", name=f"kM{i}")
                nc.sync.dma_start(t, kM[i])
                kM_sb.append(t)
                t = const.tile([PD, Bc], dt.bfloat16, tag=f"ms{i}", name=f"ms{i}")
                nc.sync.dma_start(t, msT[i])
                ms_sb.append(t)
            psT_sb = []
            for p in range(P):
                t = res.tile([PD, Bc], dt.bfloat16, tag=f"psT{p}", name=f"psT{p}")
                nc.sync.dma_start(t, psT[p])
                psT_sb.append(t)
            cload("Cg", Cg, (PD, D), dt.bfloat16)
            cload("c1", c1, (PD, 1), dt.float32)
            cload("Gm", Gm, (PD, PD), dt.float32)
            cload("grow", grow, (1, D), dt.bfloat16)
            cload("be1t", be1t, (PD, P), dt.float32)
            # prefetch first two W2' blocks behind the front-critical loads
            w2tiles = {}
            for mg in range(2):
                t = pw2.tile([PD, P, 4 * PD], dt.bfloat16, tag="w2b", name="w2b")
                nc.sync.dma_start(t, w2p[mg])
                w2tiles[mg] = t
            cload("pwt", pwt, (PD, P, PD), dt.bfloat16)
            if not zero_bias:
                cload("bprow", bprow, (1, D), dt.bfloat16)
            if not unit_ln2:
                cload("g2bc", g2bc, (PD, P, PD), dt.float32)
                cload("b2bc", b2bc, (PD, P, PD), dt.float32)

            # post-gelu activations, feature-major k-tiles (mm2 stationary)
            gsb = [
                gres.tile([PD, Bc], dt.bfloat16, tag=f"g{n}", name=f"g{n}")
                for n in range(P)
            ]

            # ---------------- front: memory read + LN1 + gelu ----------
            with tc.tile_pool(name="fr", bufs=1) as fr:
                abPM = fr.tile([PD, Bc], dt.bfloat16, tag="abPM", name="abPM")
                abF = fr.tile([PD, Bc], dt.float32, tag="abF", name="abF")
                aprime = fr.tile([PD, Bc], dt.bfloat16, tag="apr", name="apr")
                negmr = fr.tile([1, Bc], dt.bfloat16, tag="negmr", name="negmr")

                with (
                    tc.tile_pool(name="psWm", bufs=2, space="PSUM") as ppwm,
                    tc.tile_pool(name="psSp", bufs=2, space="PSUM") as ppsp,
                    tc.tile_pool(name="psBc", bufs=2, space="PSUM") as ppbc,
                    tc.tile_pool(name="psRw", bufs=2, space="PSUM") as pprw,
                ):
                    # spin the PE p-state up while input DMAs stream
                    for _ in range(5):
                        wps = ppwm.tile([PD, Bc], dt.float32, tag="wps",
                                        name="wps")
                        nc.tensor.matmul(wps, warm[:, 0:PD], warm,
                                         start=True, stop=True)

                    spM = ppsp.tile([S, Bc], dt.float32, tag="sp", name="spM")
                    nc.tensor.matmul(spM, kM_sb[0], ms_sb[0],
                                     start=True, stop=False)
                    nc.tensor.matmul(spM, kM_sb[1], ms_sb[1],
                                     start=False, stop=True)
                    spP = ppsp.tile([S, Bc], dt.float32, tag="sp", name="spP")
                    for p in range(P):
                        nc.tensor.matmul(spP, cst["kP"], psT_sb[p],
                                         start=(p == 0), stop=(p == P - 1))
                    ebM = fr.tile([S, Bc], dt.bfloat16, tag="ebM", name="ebM")
                    nc.scalar.activation(ebM, spM, AF.Exp)
                    ebP = fr.tile([S, Bc], dt.bfloat16, tag="ebP", name="ebP")
                    nc.scalar.activation(ebP, spP, AF.Exp)
                    denM = pprw.tile([1, Bc], dt.float32, tag="den", name="denM")
                    nc.tensor.matmul(denM, ones_col_b[0:S, :], ebM,
                                     start=True, stop=True)
                    denP = pprw.tile([1, Bc], dt.float32, tag="den", name="denP")
                    nc.tensor.matmul(denP, ones_col_b[0:S, :], ebP,
                                     start=True, stop=True)
                    rrM = fr.tile([1, Bc], dt.float32, tag="rrM", name="rrM")
                    nc.vector.reciprocal(rrM, denM)
                    rrP = fr.tile([1, Bc], dt.float32, tag="rrP", name="rrP")
                    nc.vector.reciprocal(rrP, denP)
                    rbcM = ppbc.tile([S, Bc], dt.float32, tag="rbc", name="rbcM")
                    nc.tensor.matmul(rbcM, ones_row_f[:, 0:S], rrM,
                                     start=True, stop=True)
                    rbcP = ppbc.tile([S, Bc], dt.float32, tag="rbc", name="rbcP")
                    nc.tensor.matmul(rbcP, ones_row_f[:, 0:S], rrP,
                                     start=True, stop=True)
                    nc.vector.tensor_mul(abPM[S:2 * S, :], ebM, rbcM)
                    nc.vector.tensor_mul(abPM[0:S, :], ebP, rbcP)

                # f32 copy of attention for exact fp32 stats matmuls
                nc.scalar.activation(abF, abPM, AF.Copy)

                with tc.tile_pool(name="psSt", bufs=1, space="PSUM") as ppst:
                    murow = ppst.tile([1, Bc], dt.float32, tag="mu", name="mu")
                    nc.tensor.matmul(murow, cst["c1"], abF, start=True, stop=True)
                    Gt = ppst.tile([PD, Bc], dt.float32, tag="Gt", name="Gt")
                    nc.tensor.matmul(Gt, cst["Gm"], abF, start=True, stop=True)
                    qq = fr.tile([PD, Bc], dt.float32, tag="qq", name="qq")
                    nc.vector.tensor_mul(qq, abF, Gt)
                    sqrow = ppst.tile([1, Bc], dt.float32, tag="sq", name="sq")
                    nc.tensor.matmul(sqrow, ones_col_f, qq, start=True, stop=True)

                    m = fr.tile([1, Bc], dt.float32, tag="m", name="m")
                    nc.vector.tensor_scalar_mul(m, murow, 1.0 / D)
                    m2 = fr.tile([1, Bc], dt.float32, tag="m2", name="m2")
                    nc.vector.tensor_mul(m2, m, m)
                    var = fr.tile([1, Bc], dt.float32, tag="var", name="var")
                    nc.vector.scalar_tensor_tensor(
                        var, sqrow, 1.0 / D, m2, op0=ALU.mult, op1=ALU.subtract
                    )
                    sd = fr.tile([1, Bc], dt.float32, tag="sd", name="sd")
                    nc.scalar.activation(sd, var, AF.Sqrt,
                                         bias=eps_t[0:1, :], scale=1.0)
                    rstd = fr.tile([1, Bc], dt.float32, tag="rstd", name="rstd")
                    nc.vector.reciprocal(rstd, sd)
                    # negmr = -mu * rstd (bf16 row, K=1 matmul operand)
                    nc.vector.scalar_tensor_tensor(
                        negmr, m, -1.0, rstd, op0=ALU.mult, op1=ALU.mult
                    )
                    rstd_bc = ppst.tile([PD, Bc], dt.float32, tag="rbc2",
                                        name="rbc2")
                    nc.tensor.matmul(rstd_bc, ones_row_f, rstd,
                                     start=True, stop=True)
                    nc.vector.tensor_mul(aprime, abPM, rstd_bc)

                # mm1': per feature tile n, LN1+gelu fused via PSUM
                with tc.tile_pool(name="psM1", bufs=2, space="PSUM") as ppm1:
                    for n in range(P):
                        nsl = slice(n * PD, (n + 1) * PD)
                        pm = ppm1.tile([PD, Bc], dt.float32, tag="pm", name="pm")
                        nc.tensor.matmul(pm, cst["grow"][:, nsl], negmr,
                                         start=True, stop=False)
                        nc.tensor.matmul(pm, cst["Cg"][:, nsl], aprime,
                                         start=False, stop=True)
                        nc.scalar.activation(
                            gsb[n], pm, AF.Gelu,
                            bias=cst["be1t"][:, n:n + 1], scale=1.0,
                        )

            # ------------- stage BC: mm2' + pair_states part + LN2 ------
            with (
                tc.tile_pool(name="yo", bufs=3) as pyo,
                tc.tile_pool(name="sc", bufs=4) as psc,
                tc.tile_pool(name="psC", bufs=3, space="PSUM") as ppc,
            ):
                for mg in range(MG):
                    w2b = w2tiles.pop(mg)
                    mgsl = slice(mg * 4 * PD, (mg + 1) * 4 * PD)
                    for bt in range(NBT):
                        bs = slice(bt * PD, (bt + 1) * PD)
                        po = ppc.tile([PD, 4 * PD], dt.float32, tag="po",
                                      name="po")
                        # the accumulation leader must write the FULL bank
                        # width with start=True: hardware start zeroes the
                        # whole 2KB PSUM zero-region, not just written cols
                        if zero_bias:
                            nc.tensor.matmul(
                                po, gsb[0][:, bs], w2b[:, 0, :],
                                start=True, stop=False,
                            )
                        else:
                            nc.tensor.matmul(
                                po, ones_row_b, cst["bprow"][:, mgsl],
                                start=True, stop=False,
                            )
                        for s in range(4):
                            pidx = 4 * mg + s
                            nc.tensor.matmul(
                                po[:, s * PD:(s + 1) * PD],
                                psT_sb[pidx][:, bs], cst["pwt"][:, pidx, :],
                                start=False, stop=False,
                                skip_group_check=True,
                            )
                        for k in range(1 if zero_bias else 0, P):
                            nc.tensor.matmul(
                                po, gsb[k][:, bs], w2b[:, k, :],
                                start=False, stop=(k == P - 1),
                                skip_group_check=True,
                            )
                        y4 = pyo.tile([PD, 4 * PD], dt.float32, tag="y4",
                                      name="y4")
                        mva = psc.tile([PD, 8], dt.float32, tag="mv", name="mv")
                        rst4 = psc.tile([PD, 4], dt.float32, tag="rst4",
                                        name="rst4")
                        for s in range(4):
                            st6 = psc.tile([PD, 6], dt.float32, tag="st6",
                                           name="st6")
                            nc.vector.bn_stats(st6, po[:, s * PD:(s + 1) * PD])
                            nc.vector.bn_aggr(
                                mva[:, 2 * s:2 * s + 2], st6)
                            sd2 = psc.tile([PD, 1], dt.float32, tag="sd2",
                                           name="sd2")
                            nc.scalar.activation(
                                sd2, mva[:, 2 * s + 1:2 * s + 2], AF.Sqrt,
                                bias=eps_t, scale=1.0)
                            nc.vector.reciprocal(rst4[:, s:s + 1], sd2)
                        for s in range(4):
                            pidx = 4 * mg + s
                            ssl = slice(s * PD, (s + 1) * PD)
                            if unit_ln2:
                                nc.vector.tensor_scalar(
                                    y4[:, ssl], po[:, ssl],
                                    mva[:, 2 * s:2 * s + 1],
                                    rst4[:, s:s + 1],
                                    op0=ALU.subtract, op1=ALU.mult,
                                )
                            else:
                                tn = psc.tile([PD, PD], dt.float32, tag="tn",
                                              name="tn")
                                nc.vector.tensor_scalar(
                                    tn, po[:, ssl],
                                    mva[:, 2 * s:2 * s + 1],
                                    rst4[:, s:s + 1],
                                    op0=ALU.subtract, op1=ALU.mult,
                                )
                                tg = psc.tile([PD, PD], dt.float32, tag="tg",
                                              name="tg")
                                nc.vector.tensor_mul(
                                    tg, tn, cst["g2bc"][:, pidx, :])
                                nc.vector.tensor_add(
                                    y4[:, ssl], tg, cst["b2bc"][:, pidx, :])
                        nc.sync.dma_start(out[bs, mgsl], y4)
                    # prefetch two blocks ahead; emitted after this block's
                    # reads so the rotated buffer is overwrite-safe
                    if mg + 2 < MG:
                        t = pw2.tile([PD, P, 4 * PD], dt.bfloat16, tag="w2b",
                                     name="w2b")
                        nc.sync.dma_start(t, w2p[mg + 2])
                        w2tiles[mg + 2] = t

    nc.compile()
    return nc


_CACHE = {}


def _get_nc(unit_ln2, zero_bias):
    key = (unit_ln2, zero_bias)
    if key not in _CACHE:
        _CACHE[key] = _build(unit_ln2, zero_bias)
    return _CACHE[key]


def _prep_in_maps(inputs):
    f32 = np.float32
    g = lambda k: np.asarray(inputs[k], f32)

    psT_full = np.asarray(g("pair_states").transpose(1, 2, 0), dtype=BF)  # [P,PD,B]
    msT_full = np.asarray(g("macro_state").T, dtype=BF)                   # [MD,B]

    W1 = g("fusion_w1")                       # (7168, 3584)
    C = np.concatenate(
        [
            g("mem_pair_vals") @ W1[:D] + g("fusion_b1")[None, :],
            g("mem_macro_vals") @ W1[D:],
        ],
        axis=0,
    )                                          # (128, 3584)
    g1 = g("fusion_ln_g")
    pw = g("pair_w")                           # (28, 256, 128)
    pwA, pwB = pw[:, :PD, :], pw[:, PD:, :]
    # W2' = W2 @ blockdiag(pwB): (3584, 28, 128)
    W2r = g("fusion_w2").reshape(D, P, PD)
    W2p = np.matmul(W2r.transpose(1, 0, 2), pwB)          # (28, 3584, 128)
    W2p = W2p.transpose(1, 0, 2).reshape(D, D)
    bp = (
        np.einsum("pc,pce->pe", g("fusion_b2").reshape(P, PD), pwB)
        + g("pair_b")
    ).reshape(1, D)

    import os
    ln2g, ln2b = g("pair_ln_g"), g("pair_ln_b")
    unit_ln2 = bool((ln2g == 1.0).all() and (ln2b == 0.0).all())
    zero_bias = bool((bp == 0.0).all())
    if os.environ.get("K_NOFAST"):
        unit_ln2 = zero_bias = False

    shared = {
        "kP": np.ascontiguousarray(
            (g("mem_pair_keys").T / (P * np.sqrt(PD))).astype(BF)),
        "kM": np.ascontiguousarray(
            (g("mem_macro_keys").T / np.sqrt(MD)).reshape(2, PD, S).astype(BF)),
        "Cg": np.ascontiguousarray((C * g1[None, :]).astype(BF)),
        "c1": np.ascontiguousarray(C.sum(axis=1, dtype=np.float64)
                                   .astype(f32).reshape(PD, 1)),
        "Gm": np.ascontiguousarray((C @ C.T).astype(f32)),
        "grow": np.ascontiguousarray(g1.reshape(1, D).astype(BF)),
        "be1t": np.ascontiguousarray(g("fusion_ln_b").reshape(P, PD).T),
        "w2p": np.ascontiguousarray(
            W2p.reshape(P, PD, MG, 4 * PD).transpose(2, 1, 0, 3).astype(BF)),
        "pwt": np.ascontiguousarray(pwA.transpose(1, 0, 2).astype(BF)),
    }
    if not zero_bias:
        shared["bprow"] = np.ascontiguousarray(bp.astype(BF))
    if not unit_ln2:
        shared["g2bc"] = np.ascontiguousarray(
            np.broadcast_to(ln2g[None], (PD, P, PD)))
        shared["b2bc"] = np.ascontiguousarray(
            np.broadcast_to(ln2b[None], (PD, P, PD)))
    in_maps = []
    for c in range(NCORES):
        m = dict(shared)
        m["psT"] = np.ascontiguousarray(psT_full[:, :, c * Bc:(c + 1) * Bc])
        m["msT"] = np.ascontiguousarray(
            msT_full[:, c * Bc:(c + 1) * Bc].reshape(2, PD, Bc))
        in_maps.append(m)
    return in_maps, unit_ln2, zero_bias


def _run(inputs, trace=False):
    in_maps, unit_ln2, zero_bias = _prep_in_maps(inputs)
    nc = _get_nc(unit_ln2, zero_bias)
    res = bass_utils.run_bass_kernel_spmd(
        nc, in_maps, core_ids=list(range(NCORES)), trace=trace
    )
    outp = np.concatenate(
        [res.results[c]["out"] for c in range(NCORES)], axis=0
    ).reshape(B, P, PD)
    return np.ascontiguousarray(outp.astype(np.float32)), res


def kernel(**inputs):
    outp, _ = _run(inputs, trace=False)
    return outp
